# revision 9
# baseline (speedup 1.0000x reference)
"""Trainium2 Bass kernel for a binarized 3-layer MLP (sign-binarize matmuls +
BatchNorm + hardtanh, final 2-class linear + log_softmax).

Strategy: pure data parallel over 8 NeuronCores (batch sharded), weights
replicated.  All matmuls contract over the feature dim, so activations are
kept feature-on-partition / batch-on-free on chip; the input x is staged
pre-transposed from the host (layout choice only - no host compute).

Since sign() values are exactly +-1, the matmuls run in fp8e4 with fp32 PSUM
accumulation (bit-exact integer dots) using DoubleRow perf mode (256-deep
contraction per pass).  BN + hardtanh + next-layer sign folds into a single
ScalarE op per tile: sign(alpha*dot + beta).  The 2-class log_softmax
collapses to out = [-softplus(d), -softplus(-d)] with d = (w4[1]-w4[0])@h3
+ (b4[1]-b4[0]), computed as one fp32r matmul with lhsT columns [+dw, -dw].
"""

import sys
import types
from contextlib import ExitStack

import numpy as np

import concourse.bacc as bacc
import concourse.bass as bass
import concourse.tile as tile
from concourse import mybir
from concourse.bass_utils import run_bass_kernel_spmd

N_CORES = 8
B = 65536
B_PC = B // N_CORES  # 8192 rows per core
IN_F = 144
H = 1152
MT = H // 128  # 9 m-tiles of 128 output features
NT = 512  # batch tile (free dim)
BN_EPS = 1e-5

F32 = mybir.dt.float32
F32R = mybir.dt.float32r
FP8 = mybir.dt.float8e4
DR = mybir.MatmulPerfMode.DoubleRow


def _install_ntff_hook():
    """Register the axon NTFF profiling hook if the image lacks
    antenv.axon_hooks (used only when tracing; harmless otherwise)."""
    try:
        import antenv
        if "antenv.axon_hooks" in sys.modules:
            return
        mod = types.ModuleType("antenv.axon_hooks")
        _h = [None]
        mod.set_axon_ntff_profile_hook = lambda h: _h.__setitem__(0, h)
        mod.get_axon_ntff_profile_hook = lambda: _h[0]
        sys.modules["antenv.axon_hooks"] = mod
        antenv.axon_hooks = mod
        from trn_agent_boot.trn_boot import _ntff_profile_via_ctypes
        mod.set_axon_ntff_profile_hook(
            _ntff_profile_via_ctypes("/opt/axon/libaxon_pjrt.so"))
    except Exception:
        pass


def build(b_pc=B_PC):
    nc = bacc.Bacc("TRN2", target_bir_lowering=False, debug=False,
                   num_devices=N_CORES)
    n_nt = b_pc // NT

    # ---- DRAM inputs (per core; weights replicated) ----
    xt = nc.dram_tensor("xt", [IN_F, b_pc], F32, kind="ExternalInput")
    w1t = nc.dram_tensor("w1t", [128, 2, H], F32, kind="ExternalInput")
    w2t = nc.dram_tensor("w2t", [128, MT, H], F32, kind="ExternalInput")
    w3t = nc.dram_tensor("w3t", [128, MT, H], F32, kind="ExternalInput")
    w4t = nc.dram_tensor("w4t", [128, MT, 2], F32, kind="ExternalInput")
    b4d = nc.dram_tensor("b4", [2], F32, kind="ExternalInput")
    cons = {}
    for l in (1, 2, 3):
        for nm in ("g", "be", "m", "v", "b"):
            cons[(nm, l)] = nc.dram_tensor(
                f"{nm}{l}r", [128, MT], F32, kind="ExternalInput")
    out = nc.dram_tensor("out", [b_pc, 2], F32, kind="ExternalOutput")

    with tile.TileContext(nc) as tc:
        with ExitStack() as ctx:
            wpool = ctx.enter_context(tc.tile_pool(name="wres", bufs=1))
            wstg = ctx.enter_context(tc.tile_pool(name="wstg", bufs=3))
            cpool = ctx.enter_context(tc.tile_pool(name="cons", bufs=1))
            xpool = ctx.enter_context(tc.tile_pool(name="xin", bufs=3))
            apool = ctx.enter_context(tc.tile_pool(name="acts", bufs=2))
            hpool = ctx.enter_context(tc.tile_pool(name="h3", bufs=2))
            opool = ctx.enter_context(tc.tile_pool(name="outs", bufs=3))
            psum = ctx.enter_context(
                tc.tile_pool(name="psum", bufs=4, space="PSUM"))
            psumd = ctx.enter_context(
                tc.tile_pool(name="psumd", bufs=2, space="PSUM"))

            # ---- one-time weight prep: sign -> fp8 residents ----
            w1s = wpool.tile([128, 2, H], FP8)
            stg = wstg.tile([128, 2 * H], F32, tag="wstg")
            nc.sync.dma_start(stg[:], w1t.ap().rearrange("p i h -> p (i h)"))
            nc.scalar.sign(w1s.rearrange("p i h -> p (i h)"), stg[:])
            w2s = wpool.tile([128, MT, H], FP8)
            w3s = wpool.tile([128, MT, H], FP8)
            for src, dst in ((w2t, w2s), (w3t, w3s)):
                for j in range(MT):
                    stg = wstg.tile([128, 2 * H], F32, tag="wstg")
                    nc.sync.dma_start(stg[:, :H], src.ap()[:, j, :])
                    nc.scalar.sign(dst[:, j, :], stg[:, :H])

            # final-layer diff weights: dw[:,:,0]=+ (w4[:,1]-w4[:,0]),
            # dw[:,:,1]=-dw[:,:,0]
            w4st = cpool.tile([128, MT, 2], F32)
            nc.sync.dma_start(w4st.rearrange("p k c -> p (k c)"),
                              w4t.ap().rearrange("p k c -> p (k c)"))
            dw = cpool.tile([128, MT, 2], F32R)
            nc.vector.tensor_tensor(
                dw[:, :, 0], w4st[:, :, 1], w4st[:, :, 0],
                mybir.AluOpType.subtract)
            nc.vector.tensor_scalar_mul(dw[:, :, 1], dw[:, :, 0], -1.0)

            # softplus bias rows: [b4[1]-b4[0], b4[0]-b4[1]]
            tb = cpool.tile([2, 1], F32)
            tbr = cpool.tile([2, 1], F32)
            nc.sync.dma_start(tb[:], b4d.ap().rearrange("(c o) -> c o", o=1))
            nc.sync.dma_start(tbr[0:1, :],
                              b4d.ap()[1:2].rearrange("(c o) -> c o", o=1))
            nc.sync.dma_start(tbr[1:2, :],
                              b4d.ap()[0:1].rearrange("(c o) -> c o", o=1))
            bias2 = cpool.tile([2, 1], F32)
            nc.vector.tensor_tensor(bias2[:], tbr[:], tb[:],
                                    mybir.AluOpType.subtract)

            # ---- BN folds: alpha = g*rsqrt(v+eps); beta = alpha*(b-m)+be ----
            alpha = {}
            beta = {}
            for l in (1, 2, 3):
                ct = {}
                for nm in ("g", "be", "m", "v", "b"):
                    t = cpool.tile([128, MT], F32, tag=f"c{nm}{l}")
                    nc.sync.dma_start(t[:], cons[(nm, l)].ap()[:, :])
                    ct[nm] = t
                a = cpool.tile([128, MT], F32, tag=f"alpha{l}")
                bt = cpool.tile([128, MT], F32, tag=f"beta{l}")
                tmp = cpool.tile([128, MT], F32, tag=f"tmp{l}")
                # tmp = sqrt(v+eps); a = 1/tmp
                nc.vector.tensor_scalar_add(tmp[:], ct["v"][:], BN_EPS)
                nc.scalar.activation(tmp[:], tmp[:],
                                     mybir.ActivationFunctionType.Sqrt)
                nc.vector.reciprocal(a[:], tmp[:])
                nc.vector.tensor_tensor(a[:], a[:], ct["g"][:],
                                        mybir.AluOpType.mult)
                # bt = a*(b-m)+be
                nc.vector.tensor_tensor(tmp[:], ct["b"][:], ct["m"][:],
                                        mybir.AluOpType.subtract)
                nc.vector.tensor_tensor(tmp[:], tmp[:], a[:],
                                        mybir.AluOpType.mult)
                nc.vector.tensor_tensor(bt[:], tmp[:], ct["be"][:],
                                        mybir.AluOpType.add)
                alpha[l] = a
                beta[l] = bt

            SIGN = mybir.ActivationFunctionType.Sign

            # ---- main batch loop ----
            for n in range(n_nt):
                ncols = bass.ts(n, NT)
                # L1 input: sign(x) tiles, feature-on-partition, padded to 256
                xa = xpool.tile([128, NT], F32, tag="xa")
                nc.sync.dma_start(xa[:], xt.ap()[0:128, ncols])
                xb = xpool.tile([16, NT], F32, tag="xb")
                nc.sync.dma_start(xb[:], xt.ap()[128:IN_F, ncols])
                a1 = apool.tile([128, 2, NT], FP8, tag="a1")
                nc.vector.memset(a1[:, 1, :], 0.0)
                nc.scalar.sign(a1[:, 0, :], xa[:])
                nc.scalar.sign(a1[0:16, 1, :], xb[:])

                # L1: one DoubleRow matmul per m-tile (K padded 144->256)
                h1 = apool.tile([128, MT, NT], FP8, tag="h1")
                for m in range(MT):
                    ps = psum.tile([128, NT], F32)
                    nc.tensor.matmul(ps[:], w1s[:, :, bass.ts(m, 128)],
                                     a1[:, :, :], start=True, stop=True,
                                     perf_mode=DR)
                    nc.scalar.activation(h1[:, m, :], ps[:], SIGN,
                                         bias=beta[1][:, m:m+1],
                                         scale=alpha[1][:, m:m+1])

                # L2 / L3
                h2 = apool.tile([128, MT, NT], FP8, tag="h2")
                h3 = hpool.tile([128, MT, NT], F32R, tag="h3")
                for l, (ws, src, dst) in (
                        (2, (w2s, h1, h2)), (3, (w3s, h2, h3))):
                    for m in range(MT):
                        ps = psum.tile([128, NT], F32)
                        mcols = bass.ts(m, 128)
                        for g in range(4):
                            nc.tensor.matmul(
                                ps[:], ws[:, 2 * g:2 * g + 2, mcols],
                                src[:, 2 * g:2 * g + 2, :],
                                start=(g == 0), stop=False, perf_mode=DR)
                        nc.tensor.matmul(ps[:], ws[:, 8, mcols],
                                         src[:, 8, :], start=False, stop=True)
                        if l == 2:
                            nc.scalar.activation(dst[:, m, :], ps[:], SIGN,
                                                 bias=beta[l][:, m:m+1],
                                                 scale=alpha[l][:, m:m+1])
                        else:
                            zc = xpool.tile([128, NT], F32, tag="zc")
                            nc.vector.tensor_scalar(
                                zc[:], ps[:],
                                alpha[l][:, m:m+1], beta[l][:, m:m+1],
                                mybir.AluOpType.mult, mybir.AluOpType.add)
                            nc.vector.tensor_scalar(
                                zc[:], zc[:], -1.0, 1.0,
                                mybir.AluOpType.max,
                                mybir.AluOpType.min)
                            nc.scalar.activation(
                                dst[:, m, :], zc[:],
                                mybir.ActivationFunctionType.Copy)

                # final: d = dw.T @ h3 (fp32r), rows [d, -d]
                dps = psumd.tile([2, NT], F32)
                for k in range(MT):
                    nc.tensor.matmul(dps[:], dw[:, k, :], h3[:, k, :],
                                     start=(k == 0), stop=(k == MT - 1))
                # out = -softplus(z) = -(relu(z) + ln(1 + exp(-|z|)))
                # per row, z = [d, -d] + bias2 rows.  Matches reference's
                # x - max - log(1 + exp(-|d|)) fp32 rounding.
                zt = opool.tile([2, NT], F32, tag="zt")
                nc.scalar.activation(zt[:], dps[:],
                                     mybir.ActivationFunctionType.Identity,
                                     bias=bias2[:, 0:1], scale=1.0)
                at = opool.tile([2, NT], F32, tag="at")
                nc.scalar.activation(at[:], zt[:],
                                     mybir.ActivationFunctionType.Abs)
                et = opool.tile([2, NT], F32, tag="et")
                nc.scalar.activation(et[:], at[:],
                                     mybir.ActivationFunctionType.Exp,
                                     scale=-1.0)
                lt = opool.tile([2, NT], F32, tag="lt")
                nc.scalar.activation(lt[:], et[:],
                                     mybir.ActivationFunctionType.Ln,
                                     bias=1.0)
                rt = opool.tile([2, NT], F32, tag="rt")
                nc.vector.tensor_scalar(rt[:], zt[:], 0.0, None,
                                        mybir.AluOpType.max)
                st = opool.tile([2, NT], F32, tag="st")
                nc.vector.tensor_tensor(st[:], rt[:], lt[:],
                                        mybir.AluOpType.add)
                ot = opool.tile([2, NT], F32, tag="ot")
                nc.vector.tensor_scalar_mul(ot[:], st[:], -1.0)
                nc.sync.dma_start(
                    out.ap()[ncols, :].rearrange("n c -> c n"), ot[:])

    nc.compile()
    return nc


_CACHE = {}


def _get_nc(b_pc):
    if b_pc not in _CACHE:
        _CACHE[b_pc] = build(b_pc)
    return _CACHE[b_pc]


def _prep_shared(w1, w2, w3, w4, b4, bn):
    """Host-side pure relayouts of weights/constants (no arithmetic on
    values; padding with zeros only)."""
    d = {}
    w1p = np.zeros((256, H), dtype=np.float32)
    w1p[:IN_F] = np.ascontiguousarray(w1.T)
    d["w1t"] = np.ascontiguousarray(w1p.reshape(2, 128, H).transpose(1, 0, 2))
    d["w2t"] = np.ascontiguousarray(
        np.ascontiguousarray(w2.T).reshape(MT, 128, H).transpose(1, 0, 2))
    d["w3t"] = np.ascontiguousarray(
        np.ascontiguousarray(w3.T).reshape(MT, 128, H).transpose(1, 0, 2))
    d["w4t"] = np.ascontiguousarray(
        np.ascontiguousarray(w4.T).reshape(MT, 128, 2).transpose(1, 0, 2))
    d["b4"] = np.ascontiguousarray(b4)
    for l in (1, 2, 3):
        for nm in ("g", "be", "m", "v", "b"):
            d[f"{nm}{l}r"] = np.ascontiguousarray(
                bn[(nm, l)].reshape(MT, 128).T)
    return d


def _run(inputs, trace=False, b_pc=B_PC):
    x = inputs["x"]
    bn = {}
    for l in (1, 2, 3):
        for nm, key in (("g", f"g{l}"), ("be", f"be{l}"), ("m", f"m{l}"),
                        ("v", f"v{l}"), ("b", f"b{l}")):
            bn[(nm, l)] = np.asarray(inputs[key], dtype=np.float32)
    shared = _prep_shared(
        np.asarray(inputs["w1"], np.float32), np.asarray(inputs["w2"], np.float32),
        np.asarray(inputs["w3"], np.float32), np.asarray(inputs["w4"], np.float32),
        np.asarray(inputs["b4"], np.float32), bn)

    xT = np.ascontiguousarray(np.asarray(x, np.float32).T)  # [144, B]
    n_use = xT.shape[1] // b_pc
    assert n_use == N_CORES, (xT.shape, b_pc)
    in_maps = []
    for c in range(N_CORES):
        m = dict(shared)
        m["xt"] = np.ascontiguousarray(xT[:, c * b_pc:(c + 1) * b_pc])
        in_maps.append(m)

    nc = _get_nc(b_pc)
    if trace:
        _install_ntff_hook()
    res = run_bass_kernel_spmd(nc, in_maps, list(range(N_CORES)), trace=trace)
    outs = [res.results[c]["out"] for c in range(N_CORES)]
    full = np.concatenate(outs, axis=0)
    return full, res.exec_time_ns


def kernel(**inputs):
    out, _ = _run(inputs, trace=False)
    return out
